# revision 1
# baseline (speedup 1.0000x reference)
"""GATv2 x2 + global mean pool on 8 Trainium2 NeuronCores (Bass/Tile).

Strategy (dst-sharded, edge-tile windows):
  - Nodes split into 8 contiguous ranges of 6250 (core k owns dst range k).
  - Edges sorted by dst; each core processes edges whose dst it owns,
    packed into tiles of <=128 edges covering <=32 whole dst nodes.
  - Per layer: node transforms are data-parallel (each core computes its
    slice of the "source" table xl and "target" table xr, both augmented
    with 4 aux columns al/ar = per-head att-weighted sums, so the leaky-relu
    score identity  sum att*leaky(S) = 0.2*(al+ar) + 0.8*sum att*relu(S)
    needs only a Relu on-chip). xl slices are AllGathered to a full table.
  - Edge phase per tile: indirect-DMA gather xl[src] rows; window xr rows
    gathered and expanded edge-wise by a one-hot matmul (I0T); S formed in
    PSUM by PE; scores via Relu + att-mul + grouped reduce; softmax without
    max-subtraction (scores are O(1)); normalization applied to edge
    weights before the scatter matmul (den -> recip -> expand via I0T);
    aggregation + bias via scatter matmul (I0) + K=1 ones matmul.
  - Layer-2 input transforms read h1 via transpose-DMA (bf16) for lhsT.
  - Global mean pool folded into per-tile matmuls with graph indicators
    pre-scaled by 1/count; final AllReduce over the 8 partial [64,256].

All tables/gathers in bf16 (f32 accumulation in PSUM), inputs/outputs f32.
"""
import sys

sys.path.insert(0, "/opt/trn_rl_repo")
sys.path.insert(0, "/opt/pypackages")

import os
from contextlib import ExitStack

import numpy as np
import ml_dtypes

import concourse.bass as bass
import concourse.mybir as mybir
import concourse.tile as tile
from concourse.bass_utils import run_bass_kernel_spmd

BF16 = ml_dtypes.bfloat16
bf = mybir.dt.bfloat16
f32 = mybir.dt.float32
i32 = mybir.dt.int32

N, E, G = 50000, 800000, 64
IN, H, D = 128, 4, 64
HD = H * D
NCORES = 8
NPC = N // NCORES            # nodes per core
P = 128                      # edge slots per tile
W = 32                       # window (dst-node) slots per tile
C = HD + 4                   # table row: 256 features + 4 aux (al/ar)
GB = 16                      # tiles per DMA batch
NCHUNK = (NPC + 1 + P - 1) // P  # 49 transform chunks (room for trash row)
NPAD = NCHUNK * P            # 6272 padded local nodes
TRASH = NPC                  # local trash row id

USE_SILU_LUT = os.environ.get("GAT_SILU_LUT", "1") == "1"

# ---------------------------------------------------------------- host prep

def _augment_w(Wm, bv, att):
    """[K,256] weight + [256] bias -> bf16 [K,260] / [1,260] with aux cols."""
    K = Wm.shape[0]
    Wa = np.zeros((K, C), np.float32)
    Wa[:, :HD] = Wm
    Wa[:, HD:] = (Wm.reshape(K, H, D) * att[None]).sum(-1)
    ba = np.zeros((1, C), np.float32)
    ba[0, :HD] = bv
    ba[0, HD:] = (bv.reshape(H, D) * att).sum(-1)
    return Wa.astype(BF16), ba.astype(BF16)


def _pack_core(src_g, dst_l, gnode, inv_cnt, T):
    """Pack one core's (dst-sorted) edges into T tiles.

    Returns srcidx [P,T] i32, winids [W,T] i32, I0 [P,T*W] bf16,
    I0T [W,T*P] bf16, gind [W,T*G] bf16.
    """
    counts = np.bincount(dst_l, minlength=NPC)
    assert counts.max() <= P, counts.max()
    starts = np.zeros(NPC + 1, np.int64)
    np.cumsum(counts, out=starts[1:])

    tiles = []  # (node_lo, node_hi)
    lo = 0
    ne = 0
    nn = 0
    for n in range(NPC):
        c = int(counts[n])
        if ne + c > P or nn + 1 > W:
            tiles.append((lo, n))
            lo, ne, nn = n, 0, 0
        ne += c
        nn += 1
    tiles.append((lo, NPC))
    t_used = len(tiles)
    assert t_used <= T, (t_used, T)

    srcidx = np.zeros((P, T), np.int32)
    winids = np.full((W, T), TRASH, np.int32)
    I0 = np.zeros((P, T, W), np.float32)
    I0T = np.zeros((W, T, P), np.float32)
    gind = np.zeros((W, T, G), np.float32)
    for t, (a, b) in enumerate(tiles):
        nw = b - a
        winids[:nw, t] = np.arange(a, b)
        gind[np.arange(nw), t, gnode[a:b]] = inv_cnt[gnode[a:b]]
        e0, e1 = starts[a], starts[b]
        k = int(e1 - e0)
        srcidx[:k, t] = src_g[e0:e1]
        offs = np.repeat(np.arange(nw), counts[a:b])
        I0[np.arange(k), t, offs] = 1.0
        I0T[offs, t, np.arange(k)] = 1.0
    return (srcidx, winids, I0.reshape(P, T * W).astype(BF16),
            I0T.reshape(W, T * P).astype(BF16),
            gind.reshape(W, T * G).astype(BF16))


def _host_prep(inputs):
    x = np.asarray(inputs["x"], np.float32)
    ei = np.asarray(inputs["edge_index"]).astype(np.int64)
    batch = np.asarray(inputs["batch"]).astype(np.int64)
    att1 = np.asarray(inputs["att1"], np.float32)
    att2 = np.asarray(inputs["att2"], np.float32)

    wl1a, blr1 = _augment_w(np.asarray(inputs["Wl1"], np.float32),
                            np.asarray(inputs["bl1"], np.float32), att1)
    wr1a, brr1 = _augment_w(np.asarray(inputs["Wr1"], np.float32),
                            np.asarray(inputs["br1"], np.float32), att1)
    wl2a, blr2 = _augment_w(np.asarray(inputs["Wl2"], np.float32),
                            np.asarray(inputs["bl2"], np.float32), att2)
    wr2a, brr2 = _augment_w(np.asarray(inputs["Wr2"], np.float32),
                            np.asarray(inputs["br2"], np.float32), att2)

    src, dst = ei[0], ei[1]
    order = np.argsort(dst, kind="stable")
    src_s, dst_s = src[order].astype(np.int32), dst[order].astype(np.int32)
    core_of = dst_s // NPC
    cnt = np.bincount(batch, minlength=G).astype(np.float32)
    inv_cnt = (1.0 / np.maximum(cnt, 1.0)).astype(np.float32)

    # common tile count T
    Ts = []
    per_core_edges = []
    for k in range(NCORES):
        m = core_of == k
        s_k, d_k = src_s[m], dst_s[m] - k * NPC
        per_core_edges.append((s_k, d_k))
        counts = np.bincount(d_k, minlength=NPC)
        lo, ne, nn, t_used = 0, 0, 0, 0
        for n in range(NPC):
            c = int(counts[n])
            if ne + c > P or nn + 1 > W:
                t_used += 1
                ne, nn = 0, 0
            ne += c
            nn += 1
        Ts.append(t_used + 1)
    T = ((max(Ts) + GB - 1) // GB) * GB

    in_maps = []
    for k in range(NCORES):
        s_k, d_k = per_core_edges[k]
        gnode = batch[k * NPC:(k + 1) * NPC].astype(np.int64)
        srcidx, winids, I0, I0T, gindm = _pack_core(s_k, d_k, gnode,
                                                    inv_cnt, T)
        xT = np.zeros((IN, NPAD), np.float32)
        xT[:, :NPC] = x[k * NPC:(k + 1) * NPC].T
        in_maps.append({
            "xT": xT.astype(BF16),
            "wl1a": wl1a, "wr1a": wr1a, "wl2a": wl2a, "wr2a": wr2a,
            "blr1": blr1, "brr1": brr1, "blr2": blr2, "brr2": brr2,
            "bias1r": np.asarray(inputs["bias1"], np.float32)[None, :]
                        .astype(BF16),
            "bias2r": np.asarray(inputs["bias2"], np.float32)[None, :]
                        .astype(BF16),
            "att1b": np.repeat(att1.reshape(1, HD), P, 0).astype(BF16),
            "att2b": np.repeat(att2.reshape(1, HD), P, 0).astype(BF16),
            "srcidx": srcidx, "winids": winids,
            "i0": I0, "i0t": I0T, "gind": gindm,
        })
    return in_maps, T

# ------------------------------------------------------------- bass program

def _legalize_waits(nc):
    """walrus allows 1 sync wait on DMA/CTRL instrs, 2 on compute instrs.
    Hoist excess waits onto same-engine NoOps inserted just before."""
    one_wait = (
        "InstDMACopy", "InstDmaTransposeAnt", "InstDMAGatherAnt",
        "InstDMAScatterAddAnt", "InstCollectiveCompute", "InstTriggerDma",
        "InstDrain", "InstNoOp", "InstEventSemaphore",
        "InstUnconditionalBranch", "InstConditionalBranch",
    )
    n_ins = 0
    for blk in nc.m.functions[0].blocks:
        out = []
        for inst in blk.instructions:
            si = inst.sync_info
            waits = list(si.on_wait) if (si is not None and si.on_wait) else []
            lim = 1  # ACT S3D3 structs also have a single wait slot
            if len(waits) > lim:
                for wchunk in waits[:-lim]:
                    nop = mybir.InstNoOp(name=f"waitnop_{n_ins}_{inst.name}",
                                         ins=[], outs=[])
                    nop.engine = inst.engine
                    nop.sync_info = mybir.SyncInfo(on_wait=[wchunk],
                                                   on_update=[])
                    out.append(nop)
                    n_ins += 1
                si.on_wait = waits[-lim:]
            out.append(inst)
        blk.instructions = out
    return n_ins


def _transform(nc, tc, ctx, lhs_src, wl, wr, blr, brr, xl_dst, xr_dst,
               transpose_in):
    """Per-chunk: xl_dst[c] = lhs_chunk @ wl + blr ; same for xr.

    lhs_src: SBUF tile [IN, NPAD] (layer 1) or DRAM h1 [NPAD, HD] with
    transpose_in=True (layer 2, K=256 via two k-chunks).
    """
    pool = ctx.enter_context(tc.tile_pool(name="tf_sb", bufs=4))
    psum = ctx.enter_context(tc.tile_pool(name="tf_ps", bufs=4, space="PSUM"))
    cpool = ctx.enter_context(tc.tile_pool(name="tf_c", bufs=1))

    kdim = HD if transpose_in else IN
    nk = kdim // P
    wlt = cpool.tile([P, nk, C], bf)
    wrt = cpool.tile([P, nk, C], bf)
    for kk in range(nk):
        nc.sync.dma_start(wlt[:, kk, :], wl[kk * P:(kk + 1) * P, :])
        nc.sync.dma_start(wrt[:, kk, :], wr[kk * P:(kk + 1) * P, :])
    blt = cpool.tile([1, C], bf)
    nc.sync.dma_start(blt[:], blr[:, :])
    brt = cpool.tile([1, C], bf)
    nc.sync.dma_start(brt[:], brr[:, :])
    ones1 = cpool.tile([1, P], bf)
    nc.gpsimd.memset(ones1[:], 1.0)

    for c in range(NCHUNK):
        if transpose_in:
            hT = pool.tile([P, nk, P], bf, tag="hT")
            for kk in range(nk):
                nc.sync.dma_start(
                    hT[:, kk, :],
                    lhs_src[c * P:(c + 1) * P, kk * P:(kk + 1) * P],
                    transpose=True)
            lhs = [hT[:, kk, :] for kk in range(nk)]
        else:
            lhs = [lhs_src[:, c * P:(c + 1) * P]]
        ps_l = psum.tile([P, C], f32, tag="psl")
        ps_r = psum.tile([P, C], f32, tag="psr")
        for kk in range(nk):
            nc.tensor.matmul(ps_l[:], lhsT=lhs[kk],
                             rhs=wlt[:, kk, :],
                             start=(kk == 0), stop=False)
        nc.tensor.matmul(ps_l[:], lhsT=ones1[:], rhs=blt[:],
                         start=False, stop=True)
        for kk in range(nk):
            nc.tensor.matmul(ps_r[:], lhsT=lhs[kk],
                             rhs=wrt[:, kk, :],
                             start=(kk == 0), stop=False)
        nc.tensor.matmul(ps_r[:], lhsT=ones1[:], rhs=brt[:],
                         start=False, stop=True)
        o_l = pool.tile([P, C], bf, tag="ol")
        nc.scalar.activation(o_l[:], ps_l[:],
                             mybir.ActivationFunctionType.Copy)
        nc.sync.dma_start(xl_dst[c * P:(c + 1) * P, :], o_l[:])
        o_r = pool.tile([P, C], bf, tag="or")
        nc.scalar.activation(o_r[:], ps_r[:],
                             mybir.ActivationFunctionType.Copy)
        nc.sync.dma_start(xr_dst[c * P:(c + 1) * P, :], o_r[:])


def _edge_layer(nc, tc, ctx, T, layer, xl_full, xr_slice, srcidx_d, winids_d,
                i0_d, i0t_d, gind_d, att_b_d, bias_row_d, h_dst, pool_ps,
                ident_bf):
    """Edge phase for one layer. layer=1: writes h to h_dst (silu'd).
    layer=2: accumulates graph pool into pool_ps."""
    pool = ctx.enter_context(tc.tile_pool(name=f"e{layer}_sb", bufs=3))
    gpool = ctx.enter_context(tc.tile_pool(name=f"e{layer}_g", bufs=2))
    psS = ctx.enter_context(tc.tile_pool(name=f"e{layer}_psS", bufs=2,
                                         space="PSUM"))
    psN = ctx.enter_context(tc.tile_pool(name=f"e{layer}_psN", bufs=2,
                                         space="PSUM"))
    psX = ctx.enter_context(tc.tile_pool(name=f"e{layer}_psX", bufs=2,
                                         space="PSUM"))
    cpool = ctx.enter_context(tc.tile_pool(name=f"e{layer}_c", bufs=1))

    att_b = cpool.tile([P, HD], bf)
    nc.sync.dma_start(att_b[:], att_b_d[:, :])
    bias_row = cpool.tile([1, HD], bf)
    nc.sync.dma_start(bias_row[:], bias_row_d[:, :])
    onesW = cpool.tile([1, W], bf)
    nc.gpsimd.memset(onesW[:], 1.0)

    NB = T // GB
    for b in range(NB):
        t0 = b * GB
        sidx = gpool.tile([P, GB], i32, tag="sidx")
        nc.sync.dma_start(sidx[:], srcidx_d[:, t0:t0 + GB])
        widx = gpool.tile([W, GB], i32, tag="widx")
        nc.sync.dma_start(widx[:], winids_d[:, t0:t0 + GB])
        i0b = gpool.tile([P, GB * W], bf, tag="i0b")
        nc.sync.dma_start(i0b[:], i0_d[:, t0 * W:(t0 + GB) * W])
        i0tb = gpool.tile([W, GB * P], bf, tag="i0tb")
        nc.sync.dma_start(i0tb[:], i0t_d[:, t0 * P:(t0 + GB) * P])
        if layer == 2:
            gindb = gpool.tile([W, GB * G], bf, tag="gindb")
            nc.sync.dma_start(gindb[:], gind_d[:, t0 * G:(t0 + GB) * G])
        xl_g = gpool.tile([P, GB, C], bf, tag="xlg")
        xr_g = gpool.tile([W, GB, C], bf, tag="xrg")
        for j in range(GB):
            nc.gpsimd.indirect_dma_start(
                out=xl_g[:, j, :], out_offset=None, in_=xl_full[:, :],
                in_offset=bass.IndirectOffsetOnAxis(
                    ap=sidx[:, j:j + 1], axis=0))
            nc.gpsimd.indirect_dma_start(
                out=xr_g[:, j, :], out_offset=None, in_=xr_slice[:, :],
                in_offset=bass.IndirectOffsetOnAxis(
                    ap=widx[:, j:j + 1], axis=0))
        if layer == 1:
            h_buf = gpool.tile([W, GB, HD], bf, tag="hbuf")

        for j in range(GB):
            i0 = i0b[:, j * W:(j + 1) * W]
            i0t = i0tb[:, j * P:(j + 1) * P]
            xl = xl_g[:, j, :]
            S = psS.tile([P, C], f32, tag="S")
            nc.tensor.matmul(S[:], lhsT=i0t, rhs=xr_g[:, j, :],
                             start=True, stop=False)
            nc.tensor.matmul(S[:], lhsT=ident_bf[:], rhs=xl,
                             start=False, stop=True)
            m = pool.tile([P, HD], bf, tag="m")
            nc.scalar.activation(m[:], S[:, :HD],
                                 mybir.ActivationFunctionType.Relu)
            wm = pool.tile([P, HD], bf, tag="wm")
            nc.vector.tensor_mul(wm[:], m[:], att_b[:])
            e0 = pool.tile([P, H], f32, tag="e0")
            nc.vector.tensor_reduce(
                out=e0[:, :, None],
                in_=wm[:].rearrange("p (h d) -> p h d", h=H),
                axis=mybir.AxisListType.X, op=mybir.AluOpType.add)
            e = pool.tile([P, H], f32, tag="e")
            nc.vector.scalar_tensor_tensor(
                out=e[:], in0=S[:, HD:HD + H], scalar=0.25,
                in1=e0[:], op0=mybir.AluOpType.mult,
                op1=mybir.AluOpType.add)
            p = pool.tile([P, H], bf, tag="p")
            nc.scalar.activation(p[:], e[:],
                                 mybir.ActivationFunctionType.Exp, scale=0.8)
            dre = psX.tile([P, 2 * H], f32, tag="dre")
            den = dre[0:W, 0:H]
            r_e = dre[:, H:2 * H]
            nc.tensor.matmul(den, lhsT=i0, rhs=p[:],
                             start=True, stop=True)
            dens = pool.tile([W, H], f32, tag="dens")
            nc.vector.tensor_scalar_add(dens[:], den, 1e-16)
            recip = pool.tile([W, H], bf, tag="recip")
            with nc.allow_low_precision(reason="attn denom O(1)"):
                nc.vector.reciprocal(recip[:], dens[:])
            nc.tensor.matmul(r_e, lhsT=i0t, rhs=recip[:],
                             start=True, stop=True)
            alpha = pool.tile([P, H], bf, tag="alpha")
            nc.vector.tensor_mul(alpha[:], p[:], r_e)
            wxl = pool.tile([P, HD], bf, tag="wxl")
            nc.vector.tensor_tensor(
                out=wxl[:].rearrange("p (h d) -> p h d", h=H),
                in0=xl[:, :HD].rearrange("p (h d) -> p h d", h=H),
                in1=alpha[:].to_broadcast([P, H, D]),
                op=mybir.AluOpType.mult)
            num = psN.tile([W, HD], f32, tag="num")
            nc.tensor.matmul(num[:], lhsT=i0, rhs=wxl[:],
                             start=True, stop=False)
            nc.tensor.matmul(num[:], lhsT=onesW[:], rhs=bias_row[:],
                             start=False, stop=True)
            if layer == 1:
                if USE_SILU_LUT:
                    nc.scalar.activation(h_buf[:, j, :], num[:],
                                         mybir.ActivationFunctionType.Silu)
                else:
                    sg = pool.tile([W, HD], bf, tag="sg")
                    nc.scalar.activation(
                        sg[:], num[:], mybir.ActivationFunctionType.Sigmoid)
                    nc.vector.tensor_mul(h_buf[:, j, :], sg[:], num[:])
            else:
                h2 = pool.tile([W, HD], bf, tag="h2")
                nc.scalar.activation(h2[:], num[:],
                                     mybir.ActivationFunctionType.Copy)
                t = t0 + j
                nc.tensor.matmul(pool_ps[:],
                                 lhsT=gindb[:, j * G:(j + 1) * G],
                                 rhs=h2[:], start=(t == 0),
                                 stop=(t == T - 1))
        if layer == 1:
            for j in range(GB):
                nc.gpsimd.indirect_dma_start(
                    out=h_dst[:, :], in_=h_buf[:, j, :],
                    out_offset=bass.IndirectOffsetOnAxis(
                        ap=widx[:, j:j + 1], axis=0),
                    in_offset=None)


def build_program(T):
    nc = bass.Bass()
    d_in = {}
    for name, shape, dt in [
        ("xT", [IN, NPAD], bf),
        ("wl1a", [IN, C], bf), ("wr1a", [IN, C], bf),
        ("wl2a", [HD, C], bf), ("wr2a", [HD, C], bf),
        ("blr1", [1, C], bf), ("brr1", [1, C], bf),
        ("blr2", [1, C], bf), ("brr2", [1, C], bf),
        ("bias1r", [1, HD], bf), ("bias2r", [1, HD], bf),
        ("att1b", [P, HD], bf), ("att2b", [P, HD], bf),
        ("srcidx", [P, T], i32), ("winids", [W, T], i32),
        ("i0", [P, T * W], bf), ("i0t", [W, T * P], bf),
        ("gind", [W, T * G], bf),
    ]:
        d_in[name] = nc.declare_dram_parameter(name, shape, dt,
                                               isOutput=False)
    out = nc.declare_dram_parameter("out", [G, HD], f32, isOutput=True)

    xl1_slice = nc.dram_tensor("xl1_slice", [NPAD, C], bf)
    xr1_slice = nc.dram_tensor("xr1_slice", [NPAD, C], bf)
    xl1_full = nc.dram_tensor("xl1_full", [N, C], bf, addr_space="Shared")
    local_h1 = nc.dram_tensor("local_h1", [NPAD, HD], bf)
    xl2_slice = nc.dram_tensor("xl2_slice", [NPAD, C], bf)
    xr2_slice = nc.dram_tensor("xr2_slice", [NPAD, C], bf)
    xl2_full = nc.dram_tensor("xl2_full", [N, C], bf, addr_space="Shared")
    pool_loc = nc.dram_tensor("pool_loc", [G, HD], f32)
    pool_sum = nc.dram_tensor("pool_sum", [G, HD], f32, addr_space="Shared")

    with tile.TileContext(nc) as tc, ExitStack() as ctx:
        from concourse.masks import make_identity
        gcpool = ctx.enter_context(tc.tile_pool(name="gc", bufs=1))
        ident = gcpool.tile([P, P], f32)
        make_identity(nc, ident[:])
        ident_bf = gcpool.tile([P, P], bf)
        nc.vector.tensor_copy(ident_bf[:], ident[:])

        # zero trash/pad rows of local_h1 (pad-slot scatters also hit TRASH,
        # but rows NPC+1.. stay unwritten and feed transpose loads)
        zrow = gcpool.tile([NPAD - NPC, HD], bf)
        nc.gpsimd.memset(zrow[:], 0.0)
        nc.sync.dma_start(local_h1[NPC:, :], zrow[:])

        # phase A: layer-1 transforms (lhsT = xT slice in SBUF)
        with ExitStack() as c1:
            xT_sb_pool = tc.tile_pool(name="xT", bufs=1)
            xTp = c1.enter_context(xT_sb_pool)
            xT_sb = xTp.tile([IN, NPAD], bf)
            nc.sync.dma_start(xT_sb[:], d_in["xT"][:, :])
            _transform(nc, tc, c1, xT_sb, d_in["wl1a"], d_in["wr1a"],
                       d_in["blr1"], d_in["brr1"], xl1_slice, xr1_slice,
                       transpose_in=False)

        # phase B: AllGather xl1
        nc.gpsimd.collective_compute(
            "AllGather", mybir.AluOpType.bypass,
            replica_groups=[list(range(NCORES))],
            ins=[xl1_slice[0:NPC, :]], outs=[xl1_full[:, :]])

        # phase C: edge layer 1
        with ExitStack() as c2:
            _edge_layer(nc, tc, c2, T, 1, xl1_full, xr1_slice,
                        d_in["srcidx"], d_in["winids"], d_in["i0"],
                        d_in["i0t"], None, d_in["att1b"], d_in["bias1r"],
                        local_h1, None, ident_bf)

        # phase D: layer-2 transforms (lhsT via transpose-DMA of h1)
        with ExitStack() as c3:
            _transform(nc, tc, c3, local_h1, d_in["wl2a"], d_in["wr2a"],
                       d_in["blr2"], d_in["brr2"], xl2_slice, xr2_slice,
                       transpose_in=True)

        # phase E: AllGather xl2
        nc.gpsimd.collective_compute(
            "AllGather", mybir.AluOpType.bypass,
            replica_groups=[list(range(NCORES))],
            ins=[xl2_slice[0:NPC, :]], outs=[xl2_full[:, :]])

        # phase F: edge layer 2 + pool accumulation
        with ExitStack() as c4:
            plp = c4.enter_context(tc.tile_pool(name="poolps", bufs=1,
                                                space="PSUM"))
            pool_ps = plp.tile([G, HD], f32)
            _edge_layer(nc, tc, c4, T, 2, xl2_full, xr2_slice,
                        d_in["srcidx"], d_in["winids"], d_in["i0"],
                        d_in["i0t"], d_in["gind"], d_in["att2b"],
                        d_in["bias2r"], None, pool_ps, ident_bf)
            psb = c4.enter_context(tc.tile_pool(name="poolsb", bufs=1))
            pool_sb = psb.tile([G, HD], f32)
            nc.scalar.activation(pool_sb[:], pool_ps[:],
                                 mybir.ActivationFunctionType.Copy)
            nc.sync.dma_start(pool_loc[:, :], pool_sb[:])
            nc.gpsimd.collective_compute(
                "AllReduce", mybir.AluOpType.add,
                replica_groups=[list(range(NCORES))],
                ins=[pool_loc[:, :]], outs=[pool_sum[:, :]])
            outt = psb.tile([G, HD], f32)
            nc.sync.dma_start(outt[:], pool_sum[:, :])
            nc.sync.dma_start(out[:, :], outt[:])

    return nc

def configure(n, g, gb=None):
    """Scale the problem down for simulator tests."""
    global N, G, NPC, NCHUNK, NPAD, TRASH, GB
    N, G = n, g
    NPC = N // NCORES
    NCHUNK = (NPC + 1 + P - 1) // P
    NPAD = NCHUNK * P
    TRASH = NPC
    if gb:
        GB = gb


# ------------------------------------------------------------------- driver

def _pjrt_prepare(nc, in_maps):
    """Build the jitted 8-core executable + device-resident inputs.

    Returns (run_fn, out_names) where run_fn() executes once (fresh donated
    zero outputs each call) and returns the concat output arrays.
    """
    import jax
    from jax.sharding import Mesh, PartitionSpec
    from jax.experimental.shard_map import shard_map
    from concourse import bass2jax

    bass2jax.install_neuronx_cc_hook()
    n_cores = len(in_maps)
    partition_name = (nc.partition_id_tensor.name
                      if nc.partition_id_tensor else None)
    in_names, out_names, out_avals, zero_outs = [], [], [], []
    for alloc in nc.m.functions[0].allocations:
        if not isinstance(alloc, mybir.MemoryLocationSet):
            continue
        name = alloc.memorylocations[0].name
        if alloc.kind == "ExternalInput":
            if name != partition_name:
                in_names.append(name)
        elif alloc.kind == "ExternalOutput":
            out_names.append(name)
            shape = tuple(alloc.tensor_shape)
            dtype = mybir.dt.np(alloc.dtype)
            out_avals.append(jax.core.ShapedArray(shape, dtype))
            zero_outs.append(np.zeros(shape, dtype))
    n_params = len(in_names)
    n_outs = len(out_avals)
    all_in_names = list(in_names) + list(out_names)
    if partition_name is not None:
        all_in_names.append(partition_name)
    donate = tuple(range(n_params, n_params + n_outs))

    def _body(*args):
        operands = list(args)
        if partition_name is not None:
            operands.append(bass2jax.partition_id_tensor())
        outs = bass2jax._bass_exec_p.bind(
            *operands,
            out_avals=tuple(out_avals),
            in_names=tuple(all_in_names),
            out_names=tuple(out_names),
            lowering_input_output_aliases=(),
            sim_require_finite=True,
            sim_require_nnan=True,
            nc=nc,
        )
        return tuple(outs)

    devices = jax.devices()[:n_cores]
    mesh = Mesh(np.asarray(devices), ("core",))
    in_specs = (PartitionSpec("core"),) * (n_params + n_outs)
    out_specs = (PartitionSpec("core"),) * len(out_names)
    sharded = jax.jit(
        shard_map(_body, mesh=mesh, in_specs=in_specs, out_specs=out_specs,
                  check_rep=False),
        keep_unused=True)
    concat_in = [
        np.concatenate([np.asarray(in_maps[c][nm]) for c in range(n_cores)],
                       axis=0)
        for nm in in_names
    ]
    from jax.sharding import NamedSharding
    sh = NamedSharding(mesh, PartitionSpec("core"))
    dev_in = [jax.device_put(a, sh) for a in concat_in]

    dev_zeros = [jax.device_put(
        np.zeros((n_cores * z.shape[0], *z.shape[1:]), z.dtype), sh)
        for z in zero_outs]

    def run_fn():
        outs = sharded(*dev_in, *dev_zeros)
        jax.block_until_ready(outs)
        return outs

    return run_fn, out_names, out_avals


def kernel(**inputs):
    in_maps, T = _host_prep(inputs)
    nc = build_program(T)
    _legalize_waits(nc)
    if os.environ.get("GAT_BENCH", "0") == "1":
        import time
        run_fn, out_names, out_avals = _pjrt_prepare(nc, in_maps)
        outs = run_fn()   # compile + first exec
        times = []
        for _ in range(int(os.environ.get("GAT_BENCH_ITERS", "5"))):
            t0 = time.perf_counter()
            outs = run_fn()
            times.append(time.perf_counter() - t0)
        kernel.last_exec_time_ns = int(min(times) * 1e9)
        kernel.bench_times = times
        i = out_names.index("out")
        full = np.asarray(outs[i]).reshape(NCORES, *out_avals[i].shape)
        return np.asarray(full[0], np.float32)
    res = run_bass_kernel_spmd(nc, in_maps, list(range(NCORES)))
    return np.asarray(res.results[0]["out"], np.float32)

